# revision 2
# baseline (speedup 1.0000x reference)
"""Contrastive loss (N=16384, D=128) on 8 TRN2 NeuronCores.

Math: with a = normalize(z1), b = normalize(z2), s = exp((a @ b.T)/tau):
  l1_i = -log(s_ii / (2*rowsum_i(s) - s_ii))
  l2_i = -log(s_ii / (2*colsum_i(s) - s_ii))      (z2/z1 swap == transpose)
  loss = mean((l1 + l2)/2)
So one pass over the NxN similarity matrix suffices: rowsums, colsums, diag.

Sharding: core k owns rows [k*2048, (k+1)*2048) of a, sees all of b.

The exp of 33.5M elements/core is the bottleneck (ACT: 1 elem/cycle/lane).
Hybrid: most tiles exp on ACT (fused rowsum via accum_out); a fraction go to
the DVE via a bf16 Schraudolph bit-trick (i16 = round(x*c1+c2) reinterpreted
as bf16 ~= exp(x/tau)), whose rowsum comes from a 4x-mode tensor_scalar with
accum_out. Column partial sums accumulate in bf16 (DVE adds) and are reduced
across partitions by PE ones-matmuls. Host: normalize, transpose, diag dots,
final log/mean in float64.
"""

import numpy as np
import ml_dtypes

N, D, NCORES = 16384, 128, 8
SHARD = N // NCORES          # 2048 a-rows per core
TAU = 0.5
EPS = 1e-12
MBS = 128                    # a-rows per block (psum partition dim)
NMB = SHARD // MBS           # 16 row blocks per core
SG = 2048                    # column stripe-group width
NSG = N // SG                # 8 stripe groups
MMN = 512                    # moving free dim per matmul (one psum bank)
NCS = N // MBS               # 128 column chunks for the colsum reduce

# Schraudolph bf16 exp: i16 = round(x*SC1 + SC2); bits viewed as bf16
# approximate exp(x/TAU). SSIG calibrated on hardware to zero the mean
# multiplicative bias of the linear-mantissa approximation.
SSIG = 0.04814
SC1 = 128.0 * np.log2(np.e) / TAU
SC2 = 128.0 * (127.0 - SSIG)

# DVE-path tile pattern: tile index t=(sg*NMB+mb) goes to the DVE when
# t % DVE_MOD == DVE_PHASE (~1/6 of tiles), balancing ACT vs DVE busy time.
DVE_MOD = 6
DVE_PHASE = 5

_cache = {}


def _is_dve_tile(sg, mb):
    return (sg * NMB + mb) % DVE_MOD == DVE_PHASE


def _fix_multiwait(nc):
    """This container's walrus accepts only ONE sync wait per instruction;
    Tile attaches several. Hoist extra waits onto single-wait NoOps placed
    just before the instruction on the same engine (engine order preserves
    semantics). DMA completion updates are never moved."""
    import concourse.mybir as mybir

    for f in nc.m.functions:
        for b in f.blocks:
            new = []
            for inst in b.instructions:
                si = inst.sync_info
                if si is not None and si.on_wait and len(si.on_wait) > 1:
                    waits = list(si.on_wait)
                    for w in waits[:-1]:
                        new.append(
                            mybir.InstNoOp(
                                name=nc.get_next_instruction_name(),
                                engine=inst.engine,
                                ins=[],
                                outs=[],
                                sync_info=mybir.SyncInfo(on_wait=[w], on_update=[]),
                            )
                        )
                    si.on_wait = [waits[-1]]
                new.append(inst)
            b.instructions = new


def _build_nc():
    from concourse import bass, tile
    import concourse.mybir as mybir

    f32 = mybir.dt.float32
    bf16 = mybir.dt.bfloat16
    i16 = mybir.dt.int16

    nc = bass.Bass()
    at_d = nc.declare_dram_parameter("at", [D, SHARD], bf16, isOutput=False)
    bt_d = nc.declare_dram_parameter("bt", [D, N], bf16, isOutput=False)
    rsa_d = nc.declare_dram_parameter("rsa", [MBS, NMB * NSG], f32, isOutput=True)
    rsd_d = nc.declare_dram_parameter("rsd", [MBS, NMB * NSG], f32, isOutput=True)
    cs_d = nc.declare_dram_parameter("cs", [MBS, NCS], f32, isOutput=True)

    CPG = SG // MBS  # colsum chunks per stripe group (16)

    with tile.TileContext(nc) as tc:
        with (
            tc.tile_pool(name="big", bufs=1) as big,
            tc.tile_pool(name="expp", bufs=6) as expp,
            tc.tile_pool(name="psum", bufs=2, space="PSUM") as psum,
        ):
            at = big.tile([D, SHARD], bf16)
            bts = [
                big.tile([D, SG], bf16, name=f"bt{sg}", tag=f"bt{sg}")
                for sg in range(NSG)
            ]
            colacc = big.tile([MBS, N], bf16)
            rsa = big.tile([MBS, NMB * NSG], f32)
            rsd = big.tile([MBS, NMB * NSG], f32)
            cs_sb = big.tile([MBS, NCS], f32)
            ones = big.tile([D, 1], bf16)
            zbias = big.tile([D, 1], f32)
            junk = big.tile([MBS, SG], bf16)

            # at + first stripe on the SP HWDGE ring (shortest critical path);
            # remaining stripes via gpsimd SWDGE queues, which round-robin
            # across DMA queues instead of serializing on the SP ring.
            nc.sync.dma_start(at[:], at_d[:])
            nc.sync.dma_start(bts[0][:], bt_d[:, 0:SG])
            for sg in range(1, NSG):
                nc.gpsimd.dma_start(bts[sg][:], bt_d[:, sg * SG:(sg + 1) * SG])
            nc.vector.memset(ones[:], 1.0)
            nc.vector.memset(zbias[:], 0.0)
            nc.vector.memset(colacc[:], 0.0)
            nc.vector.memset(rsd[:], 0.0)

            def cs_reduce(sg):
                # cs[m, sg*CPG + c] = sum_p colacc[p, (sg*CPG+c)*128 + m]
                csp = psum.tile([MBS, SG], f32, tag="mm")
                for c in range(CPG):
                    g = sg * CPG + c
                    nc.tensor.matmul(
                        csp[:, c:c + 1],
                        colacc[:, g * MBS:(g + 1) * MBS],
                        ones[:],
                        start=True,
                        stop=True,
                    )
                nc.vector.tensor_copy(
                    cs_sb[:, sg * CPG:(sg + 1) * CPG], csp[:, :CPG]
                )

            for sg in range(NSG):
                for mb in range(NMB):
                    lhs = at[:, mb * MBS:(mb + 1) * MBS]
                    ps = psum.tile([MBS, SG], f32, tag="mm")
                    for j in range(SG // MMN):
                        nc.tensor.matmul(
                            ps[:, j * MMN:(j + 1) * MMN],
                            lhs,
                            bts[sg][:, j * MMN:(j + 1) * MMN],
                            start=True,
                            stop=True,
                        )
                    ex = expp.tile([MBS, SG], bf16, tag="exp")
                    k = mb * NSG + sg
                    if _is_dve_tile(sg, mb):
                        # Schraudolph exp on the DVE: psum f32 -> bf16 bits.
                        nc.vector.tensor_scalar(
                            ex[:].bitcast(i16),
                            ps[:],
                            SC1,
                            SC2,
                            mybir.AluOpType.mult,
                            mybir.AluOpType.add,
                        )
                        # rowsum via 4x-mode tensor_scalar with accum_out
                        nc.vector.tensor_scalar(
                            junk[:],
                            ex[:],
                            1.0,
                            None,
                            mybir.AluOpType.mult,
                            mybir.AluOpType.add,
                            accum_out=rsd[:, k:k + 1],
                        )
                    else:
                        nc.scalar.activation(
                            ex[:],
                            ps[:],
                            mybir.ActivationFunctionType.Exp,
                            bias=zbias[:],
                            scale=1.0 / TAU,
                            accum_out=rsa[:, k:k + 1],
                        )
                    nc.vector.tensor_add(
                        colacc[:, sg * SG:(sg + 1) * SG],
                        colacc[:, sg * SG:(sg + 1) * SG],
                        ex[:],
                    )
                # Overlap the previous stripe's colsum partition-reduce with
                # this stripe's compute (one-stripe delay so the PE never
                # stalls on the DVE accumulation chain).
                if sg >= 1:
                    cs_reduce(sg - 1)
            cs_reduce(NSG - 1)

            nc.sync.dma_start(rsa_d[:], rsa[:])
            nc.sync.dma_start(rsd_d[:], rsd[:])
            nc.sync.dma_start(cs_d[:], cs_sb[:])

    _fix_multiwait(nc)
    return nc


def _get_nc():
    if "nc" not in _cache:
        _cache["nc"] = _build_nc()
    return _cache["nc"]


def kernel(z1, z2):
    from concourse.bass_utils import run_bass_kernel_spmd

    z1 = np.asarray(z1, dtype=np.float32)
    z2 = np.asarray(z2, dtype=np.float32)

    # Normalize in float64 (matches F.normalize: x / max(||x||, eps)).
    a64 = z1.astype(np.float64)
    b64 = z2.astype(np.float64)
    a64 /= np.maximum(np.sqrt((a64 * a64).sum(1, keepdims=True)), EPS)
    b64 /= np.maximum(np.sqrt((b64 * b64).sum(1, keepdims=True)), EPS)

    at = np.ascontiguousarray(a64.T.astype(ml_dtypes.bfloat16))   # [D, N]
    bt = np.ascontiguousarray(b64.T.astype(ml_dtypes.bfloat16))   # [D, N]

    nc = _get_nc()
    in_maps = [
        {"at": np.ascontiguousarray(at[:, k * SHARD:(k + 1) * SHARD]), "bt": bt}
        for k in range(NCORES)
    ]
    res = run_bass_kernel_spmd(
        nc, in_maps, core_ids=list(range(NCORES)), trace=_cache.get("trace", False)
    )
    _cache["last_result"] = res

    R = np.empty(N, np.float64)
    C = np.zeros(N, np.float64)
    for k in range(NCORES):
        rsk = (res.results[k]["rsa"].astype(np.float64)
               + res.results[k]["rsd"].astype(np.float64))    # [p, mb*NSG+sg]
        rsum = rsk.reshape(MBS, NMB, NSG).sum(axis=2)         # [p, mb]
        R[k * SHARD:(k + 1) * SHARD] = rsum.T.reshape(-1)     # row = mb*128+p
        csk = res.results[k]["cs"].astype(np.float64)         # [m, c] -> col c*128+m
        C += csk.T.reshape(-1)

    dot = (a64 * b64).sum(1)            # exact diag similarities
    d = np.exp(dot / TAU)
    l1 = -np.log(d / (2.0 * R - d))
    l2 = -np.log(d / (2.0 * C - d))
    loss = 0.5 * (l1 + l2).mean()
    return np.array(loss, dtype=np.float32)
